# revision 1
# baseline (speedup 1.0000x reference)
"""Causal self-attention (B=2, L=2048, E=2048, H=16, D=128) on 8 trn2 cores.

Sharding: Megatron-style tensor parallel over heads. Each core owns 2 heads:
Wq/Wk/Wv column-split, Wo row-split; x replicated (pre-transposed, bf16).
Each core computes a partial output projection; host sums partials + bias.

Device kernel per core, per (batch, head):
  - qT/kT/vT [D=128, L] built with weight slices as the stationary matmul
    operand (xT streams).  Host permutes Wq/Wk columns to (evens, odds) order
    and stacks q/k halves so RoPE is 6 contiguous [128,512] DVE ops.
  - scores per 128-row q-tile over the causal band, fp32 PSUM; diagonal block
    gets an additive -1e30 triangular mask; Exp on ACT with accum_out giving
    row sums; P normalized by reciprocal(rowsum); transposed [128,band] ->
    [128, nb, 128] with one xbar DMA-transpose; attn@v and out-projection
    accumulate in PSUM (out-proj sums the core's 2 heads).
"""

import os

import numpy as np
import ml_dtypes

import concourse.bass as bass
import concourse.tile as tile
from concourse import bacc, mybir
from concourse.bass_utils import run_bass_kernel_spmd

BF16 = mybir.dt.bfloat16
F32 = mybir.dt.float32
AF = mybir.ActivationFunctionType
ALU = mybir.AluOpType

B, L, E = 2, 2048, 2048
H, D = 16, 128
NCORES = 8
HPC = H // NCORES          # heads per core
KT = E // 128              # 16 contraction tiles
LC = L // 512              # 4 column chunks of L
QT = L // 128              # 16 q tiles
THETA = 10000.0

_PROG = None


def _build_program():
    nc = bacc.Bacc("TRN2", target_bir_lowering=False, debug=False,
                   enable_asserts=False)

    xT_d = nc.dram_tensor("xT", [B, E, L], BF16, kind="ExternalInput").ap()
    w_d = nc.dram_tensor("w", [E, HPC * 3 * 128], BF16, kind="ExternalInput").ap()
    wo_d = nc.dram_tensor("wo", [HPC, D, E], BF16, kind="ExternalInput").ap()
    cos_d = nc.dram_tensor("cosf", [128, L], F32, kind="ExternalInput").ap()
    sin_d = nc.dram_tensor("sinf", [128, L], F32, kind="ExternalInput").ap()
    tri_d = nc.dram_tensor("tri", [128, 128], F32, kind="ExternalInput").ap()
    y_d = nc.dram_tensor("y", [B, L, E], BF16, kind="ExternalOutput").ap()

    with tile.TileContext(nc) as tc:
        with tc.tile_pool(name="consts", bufs=1) as cpool, \
             tc.tile_pool(name="xt", bufs=8) as xpool, \
             tc.tile_pool(name="rope", bufs=8) as rpool, \
             tc.tile_pool(name="qkv", bufs=2) as qkvpool, \
             tc.tile_pool(name="pp", bufs=3) as ppool, \
             tc.tile_pool(name="small", bufs=8) as spool, \
             tc.tile_pool(name="outp", bufs=12) as opool, \
             tc.tile_pool(name="ys", bufs=2) as ypool, \
             tc.tile_pool(name="ps512", bufs=6, space="PSUM") as ps512, \
             tc.tile_pool(name="psy", bufs=2, space="PSUM") as psy:

            w_sb = cpool.tile([128, KT, HPC, 3, 128], BF16, tag="w")
            nc.sync.dma_start(w_sb[:], w_d.rearrange("(kt p) c -> p kt c", p=128))
            wo_sb = cpool.tile([128, HPC, E], BF16, tag="wo")
            nc.sync.dma_start(wo_sb[:], wo_d.rearrange("h p e -> p h e"))
            cosf = cpool.tile([128, L], F32, tag="cos")
            nc.sync.dma_start(cosf[:], cos_d[:])
            sinf = cpool.tile([128, L], F32, tag="sin")
            nc.sync.dma_start(sinf[:], sin_d[:])
            tri = cpool.tile([128, 128], F32, tag="tri")
            nc.sync.dma_start(tri[:], tri_d[:])

            for rep in range(int(os.environ.get("KREP", "1"))):
              for b in range(B):
                outT = [None] * HPC
                for h in range(HPC):
                    # ---- QKV projection + RoPE ----
                    qT = qkvpool.tile([128, L], BF16, tag="qT")
                    kTt = qkvpool.tile([128, L], BF16, tag="kT")
                    vTs = qkvpool.tile([128, L], BF16, tag="vT")
                    for lc in range(LC):
                        ls = lc * 512
                        pA = ps512.tile([128, 512], F32, tag="ps512")
                        pB = ps512.tile([128, 512], F32, tag="ps512")
                        pV = ps512.tile([128, 512], F32, tag="ps512")
                        for kt in range(KT):
                            xt = xpool.tile([128, 512], BF16, tag="xt")
                            nc.sync.dma_start(
                                xt[:], xT_d[b, kt * 128:(kt + 1) * 128, ls:ls + 512])
                            st = kt == 0
                            sp = kt == KT - 1
                            nc.tensor.matmul(pA[:], w_sb[:, kt, h, 0, :], xt[:],
                                             start=st, stop=sp)
                            nc.tensor.matmul(pB[:], w_sb[:, kt, h, 1, :], xt[:],
                                             start=st, stop=sp)
                            nc.tensor.matmul(pV[:], w_sb[:, kt, h, 2, :], xt[:],
                                             start=st, stop=sp)
                        # RoPE: A = [x1q;x1k], B = [x2q;x2k]
                        t1 = rpool.tile([128, 512], F32, tag="rt")
                        nc.vector.tensor_mul(t1[:], pA[:], cosf[:, ls:ls + 512])
                        t2 = rpool.tile([128, 512], F32, tag="rt")
                        nc.vector.tensor_mul(t2[:], pB[:], sinf[:, ls:ls + 512])
                        et = rpool.tile([128, 512], BF16, tag="ro")
                        nc.gpsimd.tensor_sub(et[:], t1[:], t2[:])
                        t3 = rpool.tile([128, 512], F32, tag="rt")
                        nc.vector.tensor_mul(t3[:], pA[:], sinf[:, ls:ls + 512])
                        t4 = rpool.tile([128, 512], F32, tag="rt")
                        nc.vector.tensor_mul(t4[:], pB[:], cosf[:, ls:ls + 512])
                        ot = rpool.tile([128, 512], BF16, tag="ro")
                        nc.gpsimd.tensor_add(ot[:], t3[:], t4[:])
                        nc.vector.tensor_copy(qT[0:64, ls:ls + 512], et[0:64, :])
                        nc.vector.tensor_copy(qT[64:128, ls:ls + 512], ot[0:64, :])
                        nc.vector.tensor_copy(kTt[0:64, ls:ls + 512], et[64:128, :])
                        nc.vector.tensor_copy(kTt[64:128, ls:ls + 512], ot[64:128, :])
                        nc.scalar.copy(vTs[:, ls:ls + 512], pV[:])
                    v_nat = qkvpool.tile([128, KT, 128], BF16, tag="vn")
                    nc.scalar.dma_start_transpose(out=v_nat[:], in_=vTs[:])

                    # ---- attention ----
                    outT[h] = [
                        opool.tile([128, 512], BF16, tag="outT",
                                   name=f"outT_b{b}h{h}g{g}")
                        for g in range(4)
                    ]
                    po = None
                    for i in range(QT):
                        band = (i + 1) * 128
                        nch = (band + 511) // 512
                        pt_t = ppool.tile([128, L], BF16, tag="P")
                        ds = spool.tile([128, 4], F32, tag="ds")
                        for c in range(nch):
                            c0 = c * 512
                            w = min(512, band - c0)
                            s_ps = ps512.tile([128, 512], F32, tag="ps512")
                            nc.tensor.matmul(
                                s_ps[:, 0:w], qT[:, i * 128:(i + 1) * 128],
                                kTt[:, c0:c0 + w], start=True, stop=True)
                            if c == nch - 1:
                                nc.vector.tensor_tensor(
                                    s_ps[:, w - 128:w], s_ps[:, w - 128:w],
                                    tri[:], op=ALU.add)
                            nc.scalar.activation(
                                pt_t[:, c0:c0 + w], s_ps[:, 0:w], AF.Exp,
                                accum_out=ds[:, c:c + 1])
                        dtot = spool.tile([128, 1], F32, tag="dt")
                        if nch > 1:
                            nc.vector.reduce_sum(dtot[:], ds[:, 0:nch],
                                                 axis=mybir.AxisListType.X)
                        else:
                            nc.vector.tensor_copy(dtot[:], ds[:, 0:1])
                        rinv = spool.tile([128, 1], F32, tag="ri")
                        nc.vector.reciprocal(rinv[:], dtot[:])
                        nc.vector.tensor_scalar_mul(pt_t[:, 0:band], pt_t[:, 0:band],
                                                    rinv[:])
                        ptr = ppool.tile([128, KT, 128], BF16, tag="PT")
                        nc.scalar.dma_start_transpose(out=ptr[:, 0:i + 1, :],
                                                      in_=pt_t[:, 0:band])
                        if i % 4 == 0:
                            po = ps512.tile([128, 512], F32, tag="ps512")
                        osl = (i % 4) * 128
                        for kb in range(i + 1):
                            nc.tensor.matmul(
                                po[:, osl:osl + 128], v_nat[:, kb, :],
                                ptr[:, kb, :], start=(kb == 0), stop=(kb == i))
                        if i % 4 == 3:
                            nc.scalar.copy(outT[h][i // 4][:], po[:])

                # ---- output projection (sums the core's heads) ----
                for i in range(QT):
                    ysb = ypool.tile([128, E], BF16, tag="ysb")
                    qs = (i % 4) * 128
                    for ec in range(4):
                        es = ec * 512
                        yp = psy.tile([128, 512], F32, tag="psy")
                        for h in range(HPC):
                            nc.tensor.matmul(
                                yp[:], outT[h][i // 4][:, qs:qs + 128],
                                wo_sb[:, h, es:es + 512],
                                start=(h == 0), stop=(h == HPC - 1))
                        if ec % 2 == 0:
                            nc.scalar.copy(ysb[:, es:es + 512], yp[:])
                        else:
                            nc.vector.tensor_copy(ysb[:, es:es + 512], yp[:])
                    nc.scalar.dma_start(y_d[b, i * 128:(i + 1) * 128, :], ysb[:])

    nc.compile()
    return nc


def _get_program():
    global _PROG
    if _PROG is None:
        _PROG = _build_program()
    return _PROG


def make_in_maps(x, Wq, Wk, Wv, Wo):
    """Host-side sharding/layout prep. Returns list of 8 per-core input maps."""
    bf = ml_dtypes.bfloat16
    x = np.asarray(x, np.float32)
    Wq = np.asarray(Wq, np.float32)
    Wk = np.asarray(Wk, np.float32)
    Wv = np.asarray(Wv, np.float32)
    Wo = np.asarray(Wo, np.float32)

    xT = np.ascontiguousarray(x.transpose(0, 2, 1)).astype(bf)  # [B, E, L]

    inv = THETA ** (-np.arange(0, D, 2, dtype=np.float32) / D)  # [64]
    ang = np.arange(L, dtype=np.float32)[:, None] * inv[None, :]  # [L, 64]
    cosf = np.ascontiguousarray(np.concatenate([np.cos(ang).T] * 2, axis=0)
                                ).astype(np.float32)  # [128, L]
    sinf = np.ascontiguousarray(np.concatenate([np.sin(ang).T] * 2, axis=0)
                                ).astype(np.float32)
    r = np.arange(128)
    tri = np.where(r[None, :] <= r[:, None], 0.0, -1e30).astype(np.float32)

    qscale = np.float32(D ** -0.5)
    ev = np.arange(0, D, 2)
    od = np.arange(1, D, 2)

    maps = []
    for core in range(NCORES):
        w_all = np.empty((E, HPC, 3, 128), np.float32)
        for h in range(HPC):
            g = core * HPC + h
            c0 = g * D
            w_all[:, h, 0, 0:64] = Wq[:, c0 + ev] * qscale
            w_all[:, h, 0, 64:128] = Wk[:, c0 + ev]
            w_all[:, h, 1, 0:64] = Wq[:, c0 + od] * qscale
            w_all[:, h, 1, 64:128] = Wk[:, c0 + od]
            w_all[:, h, 2, :] = Wv[:, c0:c0 + D]
        wo_c = Wo[core * HPC * D:(core + 1) * HPC * D, :].reshape(HPC, D, E)
        maps.append({
            "xT": xT,
            "w": np.ascontiguousarray(w_all.reshape(E, HPC * 3 * 128)).astype(bf),
            "wo": np.ascontiguousarray(wo_c).astype(bf),
            "cosf": cosf,
            "sinf": sinf,
            "tri": tri,
        })
    return maps


def kernel(x, Wq, Wk, Wv, Wo, bo):
    nc = _get_program()
    maps = make_in_maps(x, Wq, Wk, Wv, Wo)
    res = run_bass_kernel_spmd(nc, maps, core_ids=list(range(NCORES)))
    y = np.zeros((B, L, E), np.float64)
    for c in range(NCORES):
        y += np.asarray(res.results[c]["y"], np.float64)
    y += np.asarray(bo, np.float64)[None, None, :]
    return y.astype(np.float32)



# revision 4
# speedup vs baseline: 2.0758x; 2.0758x over previous
"""Causal self-attention (B=2, L=2048, E=2048, H=16, D=128) on 8 trn2 cores.

Sharding: Megatron-style tensor parallel over heads (2 heads/core) with
minimal host<->device traffic:
  - x is sharded: core r receives xT slice [E, 512] for (b=r//4, l-block r%4);
    an on-device AllGather rebuilds the full activation in device DRAM.
  - RoPE sin/cos tables are sharded 1/8 per core and AllGathered on device.
  - Causal masks are built on device with affine_select (no wire traffic).
  - Each core computes a full [B*L, E] partial of the output projection; an
    on-device ReduceScatter sums partials so each core returns only its
    [512, E] slice.  Host concatenates slices and adds the bias.

Device kernel per core, per batch:
  - qT/kT [D=128, L] for both heads via weight-stationary matmuls streaming
    the AllGathered xT once per batch (host pre-permutes Wq/Wk columns to
    (evens, odds) so RoPE is a handful of wide DVE/GpSimd ops).
  - attention works on transposed scores: sT[k, q] = kT.T @ qT needs no
    DMA-transposes at all; exp(sT) chunks feed attn@V (N=512 matmuls) and a
    ones-vector matmul that accumulates softmax denominators; the final
    normalization multiplies attn@V output by a PE-broadcast reciprocal.
  - out-projection contracts over D with per-head stationary tiles, writes
    bf16 partials to DRAM for the ReduceScatter.
"""

import os

import numpy as np
import ml_dtypes

import concourse.bass as bass
import concourse.tile as tile
from concourse import bacc, mybir
from concourse.bass_utils import run_bass_kernel_spmd

BF16 = mybir.dt.bfloat16
F32 = mybir.dt.float32
AF = mybir.ActivationFunctionType
ALU = mybir.AluOpType

B, L, E = 2, 2048, 2048
H, D = 16, 128
NCORES = 8
HPC = H // NCORES          # heads per core
KT = E // 128              # 16 contraction tiles
LC = L // 512              # 4 column chunks of L per batch
QT = L // 128              # 16 q tiles
QB = L // 512              # 4 q blocks of 512
THETA = 10000.0
NEG = -1.0e30

_PROG = None


def _build_program():
    nc = bacc.Bacc("TRN2", target_bir_lowering=False, debug=False,
                   enable_asserts=False, num_devices=NCORES)

    xs_d = nc.dram_tensor("xs", [E, 512], BF16, kind="ExternalInput").ap()
    tb_d = nc.dram_tensor("tb", [32, L], BF16, kind="ExternalInput").ap()
    w_d = nc.dram_tensor("w", [E, HPC * 3 * 128], BF16, kind="ExternalInput").ap()
    wo_d = nc.dram_tensor("wo", [HPC * D, E], BF16, kind="ExternalInput").ap()
    y_d = nc.dram_tensor("y", [B * L // NCORES, E], BF16, kind="ExternalOutput").ap()

    grp = [list(range(NCORES))]

    with tile.TileContext(nc) as tc:
        with tc.tile_pool(name="consts", bufs=1) as cpool, \
             tc.tile_pool(name="xt", bufs=8) as xpool, \
             tc.tile_pool(name="rope", bufs=8) as rpool, \
             tc.tile_pool(name="qkv", bufs=4) as qkvpool, \
             tc.tile_pool(name="pp", bufs=8) as ppool, \
             tc.tile_pool(name="small", bufs=8) as spool, \
             tc.tile_pool(name="rbs", bufs=2) as rbpool, \
             tc.tile_pool(name="ys", bufs=2) as ypool, \
             tc.tile_pool(name="ps", bufs=8, space="PSUM") as pspool, \
             tc.tile_pool(name="dram", bufs=1, space="DRAM") as dram:

            # ---- weights ----
            w_sb = cpool.tile([128, KT, HPC, 3, 128], BF16, tag="w")
            nc.sync.dma_start(w_sb[:], w_d.rearrange("(kt p) c -> p kt c", p=128))
            wo_sb = cpool.tile([128, HPC, E], BF16, tag="wo")
            nc.sync.dma_start(wo_sb[:], wo_d.rearrange("(h p) e -> p h e", p=128))

            # ---- RoPE tables: AllGather 1/8 shards, convert to f32 ----
            tb_b = dram.tile([32, L], BF16, tag="tb_b")
            tblg = dram.tile([32 * NCORES, L], BF16, tag="tblg")
            nc.gpsimd.dma_start(tb_b[:], tb_d[:])
            nc.gpsimd.collective_compute(
                "AllGather", ALU.bypass, replica_groups=grp,
                ins=[tb_b.opt()], outs=[tblg.opt()])
            cs = cpool.tile([128, L], F32, tag="cs")
            ss = cpool.tile([128, L], F32, tag="ss")
            for dst, r0 in ((cs, 0), (ss, 128)):
                for c4 in range(4):
                    tmp = xpool.tile([128, 512], BF16, tag="xt",
                                     name=f"tbl_{r0}_{c4}")
                    nc.sync.dma_start(tmp[:], tblg[r0:r0 + 128,
                                                   c4 * 512:(c4 + 1) * 512])
                    nc.vector.tensor_copy(dst[:, c4 * 512:(c4 + 1) * 512], tmp[:])

            # ---- causal masks for the 4 diagonal chunk offsets ----
            masks = []
            for d in range(4):
                m = cpool.tile([128, 512], F32, tag=f"mask{d}")
                nc.gpsimd.memset(m[:], 0.0)
                # keep 0 where (col - part - 128*d) >= 0, else -1e30
                nc.gpsimd.affine_select(
                    m[:], m[:], pattern=[[1, 512]], compare_op=ALU.is_ge,
                    fill=NEG, base=-128 * d, channel_multiplier=-1)
                masks.append(m)

            ones_col = cpool.tile([128, 1], BF16, tag="ones_col")
            nc.gpsimd.memset(ones_col[:], 1.0)
            ones_row = cpool.tile([1, 128], BF16, tag="ones_row")
            nc.gpsimd.memset(ones_row[:], 1.0)

            for rep in range(int(os.environ.get("KREP", "1"))):
                # ---- AllGather x shards ----
                xs_b = dram.tile([E, 512], BF16, tag="xs_b")
                xg = dram.tile([NCORES * E, 512], BF16, tag="xg")
                nc.gpsimd.dma_start(xs_b[:], xs_d[:])
                nc.gpsimd.collective_compute(
                    "AllGather", ALU.bypass, replica_groups=grp,
                    ins=[xs_b.opt()], outs=[xg.opt()])

                ypart = dram.tile([B * L, E], BF16, tag="ypart")

                for b in range(B):
                    # ---- QKV projection + RoPE for both heads ----
                    qT = [qkvpool.tile([128, L], BF16, tag="qT",
                                       name=f"qT_b{b}h{h}") for h in range(HPC)]
                    kT = [qkvpool.tile([128, L], BF16, tag="kT",
                                       name=f"kT_b{b}h{h}") for h in range(HPC)]
                    vTs = [qkvpool.tile([128, L], BF16, tag="vTs",
                                        name=f"vTs_b{b}h{h}") for h in range(HPC)]
                    for lc in range(LC):
                        ls = lc * 512
                        pA = [pspool.tile([128, 512], F32, tag="ps",
                                          name=f"pA{b}_{lc}_{h}") for h in range(HPC)]
                        pB = [pspool.tile([128, 512], F32, tag="ps",
                                          name=f"pB{b}_{lc}_{h}") for h in range(HPC)]
                        pV = [pspool.tile([128, 512], F32, tag="ps",
                                          name=f"pV{b}_{lc}_{h}") for h in range(HPC)]
                        for kt in range(KT):
                            xt = xpool.tile([128, 512], BF16, tag="xt")
                            row0 = (b * LC + lc) * E + kt * 128
                            nc.sync.dma_start(xt[:], xg[row0:row0 + 128, :])
                            st = kt == 0
                            sp = kt == KT - 1
                            for h in range(HPC):
                                nc.tensor.matmul(pA[h][:], w_sb[:, kt, h, 0, :], xt[:],
                                                 start=st, stop=sp)
                                nc.tensor.matmul(pB[h][:], w_sb[:, kt, h, 1, :], xt[:],
                                                 start=st, stop=sp)
                                nc.tensor.matmul(pV[h][:], w_sb[:, kt, h, 2, :], xt[:],
                                                 start=st, stop=sp)
                        for h in range(HPC):
                            # RoPE: rows of A/B are [q-even|k-even] / [q-odd|k-odd]
                            t1 = rpool.tile([128, 512], F32, tag="rt")
                            nc.vector.tensor_mul(t1[:], pA[h][:], cs[:, ls:ls + 512])
                            t2 = rpool.tile([128, 512], F32, tag="rt")
                            nc.vector.tensor_mul(t2[:], pB[h][:], ss[:, ls:ls + 512])
                            t3 = rpool.tile([128, 512], F32, tag="rt")
                            nc.vector.tensor_mul(t3[:], pA[h][:], ss[:, ls:ls + 512])
                            t4 = rpool.tile([128, 512], F32, tag="rt")
                            nc.vector.tensor_mul(t4[:], pB[h][:], cs[:, ls:ls + 512])
                            nc.gpsimd.tensor_sub(qT[h][0:64, ls:ls + 512],
                                                 t1[0:64, :], t2[0:64, :])
                            nc.gpsimd.tensor_sub(kT[h][0:64, ls:ls + 512],
                                                 t1[64:128, :], t2[64:128, :])
                            nc.gpsimd.tensor_add(qT[h][64:128, ls:ls + 512],
                                                 t3[0:64, :], t4[0:64, :])
                            nc.gpsimd.tensor_add(kT[h][64:128, ls:ls + 512],
                                                 t3[64:128, :], t4[64:128, :])
                            nc.scalar.copy(vTs[h][:, ls:ls + 512], pV[h][:])

                    vN = [qkvpool.tile([128, KT, 128], BF16, tag="vN",
                                       name=f"vN_b{b}h{h}") for h in range(HPC)]
                    for h in range(HPC):
                        nc.scalar.dma_start_transpose(out=vN[h][:], in_=vTs[h][:])

                    # ---- attention (transposed scores; no P transposes) ----
                    outT = [qkvpool.tile([128, L], BF16, tag="oT",
                                         name=f"oT_b{b}h{h}") for h in range(HPC)]
                    for h in range(HPC):
                        for qb in range(QB):
                            qs = qb * 512
                            nch = 4 * (qb + 1)
                            pv = pspool.tile([128, 512], F32, tag="ps")
                            dn = pspool.tile([1, 512], F32, tag="ps")
                            for kb in range(nch):
                                s = pspool.tile([128, 512], F32, tag="ps")
                                nc.tensor.matmul(
                                    s[:], kT[h][:, kb * 128:(kb + 1) * 128],
                                    qT[h][:, qs:qs + 512], start=True, stop=True)
                                dd = kb - 4 * qb
                                if dd >= 0:
                                    nc.vector.tensor_tensor(
                                        s[:], s[:], masks[dd][:], op=ALU.add)
                                pt = ppool.tile([128, 512], BF16, tag="pt")
                                nc.scalar.activation(pt[:], s[:], AF.Exp)
                                st = kb == 0
                                sp = kb == nch - 1
                                nc.tensor.matmul(dn[:], ones_col[:], pt[:],
                                                 start=st, stop=sp)
                                nc.tensor.matmul(pv[:], vN[h][:, kb, :], pt[:],
                                                 start=st, stop=sp)
                            rinv = spool.tile([1, 512], F32, tag="ri")
                            nc.vector.reciprocal(rinv[:], dn[:])
                            rinv_h = spool.tile([1, 512], BF16, tag="rih")
                            nc.vector.tensor_copy(rinv_h[:], rinv[:])
                            rb = pspool.tile([128, 512], F32, tag="ps")
                            nc.tensor.matmul(rb[:], ones_row[:], rinv_h[:],
                                             start=True, stop=True)
                            rbs = rbpool.tile([128, 512], F32, tag="rbs")
                            nc.scalar.copy(rbs[:], rb[:])
                            nc.vector.tensor_mul(outT[h][:, qs:qs + 512],
                                                 pv[:], rbs[:])

                    # ---- output projection (partial over this core's heads) ----
                    for qt in range(QT):
                        ysb = ypool.tile([128, E], BF16, tag="ysb")
                        qs = qt * 128
                        yp = [pspool.tile([128, 512], F32, tag="ps",
                                          name=f"yp{b}_{qt}_{ec}") for ec in range(4)]
                        for h in range(HPC):
                            for ec in range(4):
                                nc.tensor.matmul(
                                    yp[ec][:], outT[h][:, qs:qs + 128],
                                    wo_sb[:, h, ec * 512:(ec + 1) * 512],
                                    start=(h == 0), stop=(h == HPC - 1))
                        for ec in range(4):
                            es = ec * 512
                            if ec % 2 == 0:
                                nc.scalar.copy(ysb[:, es:es + 512], yp[ec][:])
                            else:
                                nc.vector.tensor_copy(ysb[:, es:es + 512], yp[ec][:])
                        nc.sync.dma_start(
                            ypart[b * L + qs:b * L + qs + 128, :], ysb[:])

                # ---- ReduceScatter partials; write this core's slice ----
                ysl = dram.tile([B * L // NCORES, E], BF16, tag="ysl")
                nc.gpsimd.collective_compute(
                    "ReduceScatter", ALU.add, replica_groups=grp,
                    ins=[ypart.opt()], outs=[ysl.opt()])
                nc.gpsimd.dma_start(y_d[:], ysl[:])

    nc.compile()
    return nc


def _get_program():
    global _PROG
    if _PROG is None:
        _PROG = _build_program()
    return _PROG


def make_in_maps(x, Wq, Wk, Wv, Wo):
    """Host-side sharding/layout prep. Returns list of 8 per-core input maps."""
    bf = ml_dtypes.bfloat16
    x = np.asarray(x, np.float32)
    Wq = np.asarray(Wq, np.float32)
    Wk = np.asarray(Wk, np.float32)
    Wv = np.asarray(Wv, np.float32)
    Wo = np.asarray(Wo, np.float32)

    xT = np.ascontiguousarray(x.transpose(0, 2, 1)).astype(bf)  # [B, E, L]

    inv = THETA ** (-np.arange(0, D, 2, dtype=np.float32) / D)  # [64]
    ang = np.arange(L, dtype=np.float32)[:, None] * inv[None, :]  # [L, 64]
    cosf = np.concatenate([np.cos(ang).T] * 2, axis=0)  # [128, L]
    sinf = np.concatenate([np.sin(ang).T] * 2, axis=0)
    tbl = np.ascontiguousarray(
        np.concatenate([cosf, sinf], axis=0)).astype(bf)  # [256, L]

    qscale = np.float32(D ** -0.5)
    ev = np.arange(0, D, 2)
    od = np.arange(1, D, 2)

    maps = []
    for core in range(NCORES):
        w_all = np.empty((E, HPC, 3, 128), np.float32)
        for h in range(HPC):
            g = core * HPC + h
            c0 = g * D
            w_all[:, h, 0, 0:64] = Wq[:, c0 + ev] * qscale
            w_all[:, h, 0, 64:128] = Wk[:, c0 + ev]
            w_all[:, h, 1, 0:64] = Wq[:, c0 + od] * qscale
            w_all[:, h, 1, 64:128] = Wk[:, c0 + od]
            w_all[:, h, 2, :] = Wv[:, c0:c0 + D]
        wo_c = Wo[core * HPC * D:(core + 1) * HPC * D, :]
        b_r, l_r = core // LC, (core % LC) * 512
        maps.append({
            "xs": np.ascontiguousarray(xT[b_r, :, l_r:l_r + 512]),
            "tb": np.ascontiguousarray(tbl[core * 32:(core + 1) * 32, :]),
            "w": np.ascontiguousarray(w_all.reshape(E, HPC * 3 * 128)).astype(bf),
            "wo": np.ascontiguousarray(wo_c).astype(bf),
        })
    return maps


def kernel(x, Wq, Wk, Wv, Wo, bo):
    nc = _get_program()
    maps = make_in_maps(x, Wq, Wk, Wv, Wo)
    res = run_bass_kernel_spmd(nc, maps, core_ids=list(range(NCORES)))
    y = np.concatenate(
        [np.asarray(res.results[c]["y"], np.float32) for c in range(NCORES)],
        axis=0).reshape(B, L, E)
    y += np.asarray(bo, np.float32)[None, None, :]
    return y.astype(np.float32)


# revision 5
# speedup vs baseline: 2.1728x; 1.0468x over previous
"""Causal self-attention (B=2, L=2048, E=2048, H=16, D=128) on 8 trn2 cores.

Sharding: Megatron-style tensor parallel over heads (2 heads/core) with
minimal host<->device traffic:
  - x is sharded: core r receives xT slice [E, 512] for (b=r//4, l-block r%4);
    an on-device AllGather rebuilds the full activation in device DRAM.
  - RoPE sin/cos tables are sharded 1/8 per core and AllGathered on device.
  - Causal masks are built on device with affine_select (no wire traffic).
  - Each core computes a full [B*L, E] partial of the output projection; an
    on-device ReduceScatter sums partials so each core returns only its
    [512, E] slice.  Host concatenates slices and adds the bias.

Device kernel per core, per batch:
  - qT/kT [D=128, L] for both heads via weight-stationary matmuls streaming
    the AllGathered xT once per batch (host pre-permutes Wq/Wk columns to
    (evens, odds) so RoPE is a handful of wide DVE/GpSimd ops).
  - attention works on transposed scores: sT[k, q] = kT.T @ qT needs no
    DMA-transposes at all; exp(sT) chunks feed attn@V (N=512 matmuls) and a
    ones-vector matmul that accumulates softmax denominators; the final
    normalization multiplies attn@V output by a PE-broadcast reciprocal.
  - out-projection contracts over D with per-head stationary tiles, writes
    bf16 partials to DRAM for the ReduceScatter.
"""

import os

import numpy as np
import ml_dtypes

import concourse.bass as bass
import concourse.tile as tile
from concourse import bacc, mybir
from concourse.bass_utils import run_bass_kernel_spmd

BF16 = mybir.dt.bfloat16
F32 = mybir.dt.float32
AF = mybir.ActivationFunctionType
ALU = mybir.AluOpType

B, L, E = 2, 2048, 2048
H, D = 16, 128
NCORES = 8
HPC = H // NCORES          # heads per core
KT = E // 128              # 16 contraction tiles
LC = L // 512              # 4 column chunks of L per batch
QT = L // 128              # 16 q tiles
QB = L // 512              # 4 q blocks of 512
THETA = 10000.0
NEG = -1.0e30

_PROG = None


def _build_program():
    nc = bacc.Bacc("TRN2", target_bir_lowering=False, debug=False,
                   enable_asserts=False, num_devices=NCORES)

    # single packed input: rows [0:2048] xT slice, [2048:2176] rope-table
    # shard, [2176:5248] qkv weights ([p, kt, h, j, d] order), [5248:6272]
    # out-proj weights ([p, h, e] order)
    pk_d = nc.dram_tensor("pk", [6272, 512], BF16, kind="ExternalInput").ap()
    y_d = nc.dram_tensor("y", [B * L // NCORES, E], BF16, kind="ExternalOutput").ap()

    grp = [list(range(NCORES))]

    with tile.TileContext(nc) as tc:
        with tc.tile_pool(name="consts", bufs=1) as cpool, \
             tc.tile_pool(name="xt", bufs=8) as xpool, \
             tc.tile_pool(name="rope", bufs=8) as rpool, \
             tc.tile_pool(name="qkv", bufs=4) as qkvpool, \
             tc.tile_pool(name="pp", bufs=8) as ppool, \
             tc.tile_pool(name="small", bufs=8) as spool, \
             tc.tile_pool(name="rbs", bufs=2) as rbpool, \
             tc.tile_pool(name="ys", bufs=2) as ypool, \
             tc.tile_pool(name="ps", bufs=8, space="PSUM") as pspool, \
             tc.tile_pool(name="dram", bufs=1, space="DRAM") as dram:

            # ---- weights ----
            w_sb = cpool.tile([128, KT, HPC, 3, 128], BF16, tag="w")
            nc.sync.dma_start(
                w_sb[:], pk_d[2176:5248, :].rearrange("(p r) c -> p (r c)", p=128))
            wo_sb = cpool.tile([128, HPC, E], BF16, tag="wo")
            nc.sync.dma_start(
                wo_sb[:], pk_d[5248:6272, :].rearrange("(p r) c -> p (r c)", p=128))

            # ---- RoPE tables: AllGather 1/8 shards, convert to f32 ----
            tb_b = dram.tile([32, L], BF16, tag="tb_b")
            tblg = dram.tile([32 * NCORES, L], BF16, tag="tblg")
            nc.gpsimd.dma_start(tb_b[:], pk_d[2048:2176, :])
            nc.gpsimd.collective_compute(
                "AllGather", ALU.bypass, replica_groups=grp,
                ins=[tb_b.opt()], outs=[tblg.opt()])
            cs = cpool.tile([128, L], F32, tag="cs")
            ss = cpool.tile([128, L], F32, tag="ss")
            for dst, r0 in ((cs, 0), (ss, 128)):
                for c4 in range(4):
                    tmp = xpool.tile([128, 512], BF16, tag="xt",
                                     name=f"tbl_{r0}_{c4}")
                    nc.sync.dma_start(tmp[:], tblg[r0:r0 + 128,
                                                   c4 * 512:(c4 + 1) * 512])
                    nc.vector.tensor_copy(dst[:, c4 * 512:(c4 + 1) * 512], tmp[:])

            # ---- causal masks for the 4 diagonal chunk offsets ----
            masks = []
            for d in range(4):
                m = cpool.tile([128, 512], F32, tag=f"mask{d}")
                nc.gpsimd.memset(m[:], 0.0)
                # keep 0 where (col - part - 128*d) >= 0, else -1e30
                nc.gpsimd.affine_select(
                    m[:], m[:], pattern=[[1, 512]], compare_op=ALU.is_ge,
                    fill=NEG, base=-128 * d, channel_multiplier=-1)
                masks.append(m)

            ones_col = cpool.tile([128, 1], BF16, tag="ones_col")
            nc.gpsimd.memset(ones_col[:], 1.0)
            ones_row = cpool.tile([1, 128], BF16, tag="ones_row")
            nc.gpsimd.memset(ones_row[:], 1.0)

            for rep in range(int(os.environ.get("KREP", "1"))):
                # ---- AllGather x shards ----
                xs_b = dram.tile([E, 512], BF16, tag="xs_b")
                xg = dram.tile([NCORES * E, 512], BF16, tag="xg")
                nc.gpsimd.dma_start(xs_b[:], pk_d[0:E, :])
                nc.gpsimd.collective_compute(
                    "AllGather", ALU.bypass, replica_groups=grp,
                    ins=[xs_b.opt()], outs=[xg.opt()])

                ypart = dram.tile([B * L, E], BF16, tag="ypart")

                for b in range(B):
                    # ---- QKV projection + RoPE for both heads ----
                    qT = [qkvpool.tile([128, L], BF16, tag="qT",
                                       name=f"qT_b{b}h{h}") for h in range(HPC)]
                    kT = [qkvpool.tile([128, L], BF16, tag="kT",
                                       name=f"kT_b{b}h{h}") for h in range(HPC)]
                    vTs = [qkvpool.tile([128, L], BF16, tag="vTs",
                                        name=f"vTs_b{b}h{h}") for h in range(HPC)]
                    for lc in range(LC):
                        ls = lc * 512
                        pA = [pspool.tile([128, 512], F32, tag="ps",
                                          name=f"pA{b}_{lc}_{h}") for h in range(HPC)]
                        pB = [pspool.tile([128, 512], F32, tag="ps",
                                          name=f"pB{b}_{lc}_{h}") for h in range(HPC)]
                        pV = [pspool.tile([128, 512], F32, tag="ps",
                                          name=f"pV{b}_{lc}_{h}") for h in range(HPC)]
                        for kt in range(KT):
                            xt = xpool.tile([128, 512], BF16, tag="xt")
                            row0 = (b * LC + lc) * E + kt * 128
                            nc.sync.dma_start(xt[:], xg[row0:row0 + 128, :])
                            st = kt == 0
                            sp = kt == KT - 1
                            for h in range(HPC):
                                nc.tensor.matmul(pA[h][:], w_sb[:, kt, h, 0, :], xt[:],
                                                 start=st, stop=sp)
                                nc.tensor.matmul(pB[h][:], w_sb[:, kt, h, 1, :], xt[:],
                                                 start=st, stop=sp)
                                nc.tensor.matmul(pV[h][:], w_sb[:, kt, h, 2, :], xt[:],
                                                 start=st, stop=sp)
                        for h in range(HPC):
                            # RoPE: rows of A/B are [q-even|k-even] / [q-odd|k-odd]
                            t1 = rpool.tile([128, 512], F32, tag="rt")
                            nc.vector.tensor_mul(t1[:], pA[h][:], cs[:, ls:ls + 512])
                            t2 = rpool.tile([128, 512], F32, tag="rt")
                            nc.vector.tensor_mul(t2[:], pB[h][:], ss[:, ls:ls + 512])
                            t3 = rpool.tile([128, 512], F32, tag="rt")
                            nc.vector.tensor_mul(t3[:], pA[h][:], ss[:, ls:ls + 512])
                            t4 = rpool.tile([128, 512], F32, tag="rt")
                            nc.vector.tensor_mul(t4[:], pB[h][:], cs[:, ls:ls + 512])
                            nc.gpsimd.tensor_sub(qT[h][0:64, ls:ls + 512],
                                                 t1[0:64, :], t2[0:64, :])
                            nc.gpsimd.tensor_sub(kT[h][0:64, ls:ls + 512],
                                                 t1[64:128, :], t2[64:128, :])
                            nc.gpsimd.tensor_add(qT[h][64:128, ls:ls + 512],
                                                 t3[0:64, :], t4[0:64, :])
                            nc.gpsimd.tensor_add(kT[h][64:128, ls:ls + 512],
                                                 t3[64:128, :], t4[64:128, :])
                            nc.scalar.copy(vTs[h][:, ls:ls + 512], pV[h][:])

                    vN = [qkvpool.tile([128, KT, 128], BF16, tag="vN",
                                       name=f"vN_b{b}h{h}") for h in range(HPC)]
                    for h in range(HPC):
                        nc.scalar.dma_start_transpose(out=vN[h][:], in_=vTs[h][:])

                    # ---- attention (transposed scores; no P transposes) ----
                    outT = [qkvpool.tile([128, L], BF16, tag="oT",
                                         name=f"oT_b{b}h{h}") for h in range(HPC)]
                    for h in range(HPC):
                        for qb in range(QB):
                            qs = qb * 512
                            nch = 4 * (qb + 1)
                            pv = pspool.tile([128, 512], F32, tag="ps")
                            dn = pspool.tile([1, 512], F32, tag="ps")
                            for kb in range(nch):
                                s = pspool.tile([128, 512], F32, tag="ps")
                                nc.tensor.matmul(
                                    s[:], kT[h][:, kb * 128:(kb + 1) * 128],
                                    qT[h][:, qs:qs + 512], start=True, stop=True)
                                dd = kb - 4 * qb
                                if dd >= 0:
                                    nc.vector.tensor_tensor(
                                        s[:], s[:], masks[dd][:], op=ALU.add)
                                pt = ppool.tile([128, 512], BF16, tag="pt")
                                nc.scalar.activation(pt[:], s[:], AF.Exp)
                                st = kb == 0
                                sp = kb == nch - 1
                                nc.tensor.matmul(dn[:], ones_col[:], pt[:],
                                                 start=st, stop=sp)
                                nc.tensor.matmul(pv[:], vN[h][:, kb, :], pt[:],
                                                 start=st, stop=sp)
                            rinv = spool.tile([1, 512], F32, tag="ri")
                            nc.vector.reciprocal(rinv[:], dn[:])
                            rinv_h = spool.tile([1, 512], BF16, tag="rih")
                            nc.vector.tensor_copy(rinv_h[:], rinv[:])
                            rb = pspool.tile([128, 512], F32, tag="ps")
                            nc.tensor.matmul(rb[:], ones_row[:], rinv_h[:],
                                             start=True, stop=True)
                            rbs = rbpool.tile([128, 512], F32, tag="rbs")
                            nc.scalar.copy(rbs[:], rb[:])
                            nc.vector.tensor_mul(outT[h][:, qs:qs + 512],
                                                 pv[:], rbs[:])

                    # ---- output projection (partial over this core's heads) ----
                    for qt in range(QT):
                        ysb = ypool.tile([128, E], BF16, tag="ysb")
                        qs = qt * 128
                        yp = [pspool.tile([128, 512], F32, tag="ps",
                                          name=f"yp{b}_{qt}_{ec}") for ec in range(4)]
                        for h in range(HPC):
                            for ec in range(4):
                                nc.tensor.matmul(
                                    yp[ec][:], outT[h][:, qs:qs + 128],
                                    wo_sb[:, h, ec * 512:(ec + 1) * 512],
                                    start=(h == 0), stop=(h == HPC - 1))
                        for ec in range(4):
                            es = ec * 512
                            if ec % 2 == 0:
                                nc.scalar.copy(ysb[:, es:es + 512], yp[ec][:])
                            else:
                                nc.vector.tensor_copy(ysb[:, es:es + 512], yp[ec][:])
                        nc.sync.dma_start(
                            ypart[b * L + qs:b * L + qs + 128, :], ysb[:])

                # ---- ReduceScatter partials; write this core's slice ----
                ysl = dram.tile([B * L // NCORES, E], BF16, tag="ysl")
                nc.gpsimd.collective_compute(
                    "ReduceScatter", ALU.add, replica_groups=grp,
                    ins=[ypart.opt()], outs=[ysl.opt()])
                nc.gpsimd.dma_start(y_d[:], ysl[:])

    nc.compile()
    return nc


def _get_program():
    global _PROG
    if _PROG is None:
        _PROG = _build_program()
    return _PROG


def make_in_maps(x, Wq, Wk, Wv, Wo):
    """Host-side sharding/layout prep. Returns list of 8 per-core input maps."""
    bf = ml_dtypes.bfloat16
    x = np.asarray(x, np.float32)
    Wq = np.asarray(Wq, np.float32)
    Wk = np.asarray(Wk, np.float32)
    Wv = np.asarray(Wv, np.float32)
    Wo = np.asarray(Wo, np.float32)

    xT = np.ascontiguousarray(x.transpose(0, 2, 1)).astype(bf)  # [B, E, L]

    inv = THETA ** (-np.arange(0, D, 2, dtype=np.float32) / D)  # [64]
    ang = np.arange(L, dtype=np.float32)[:, None] * inv[None, :]  # [L, 64]
    cosf = np.concatenate([np.cos(ang).T] * 2, axis=0)  # [128, L]
    sinf = np.concatenate([np.sin(ang).T] * 2, axis=0)
    tbl = np.ascontiguousarray(
        np.concatenate([cosf, sinf], axis=0)).astype(bf)  # [256, L]

    qscale = np.float32(D ** -0.5)
    ev = np.arange(0, D, 2)
    od = np.arange(1, D, 2)

    maps = []
    for core in range(NCORES):
        w_all = np.empty((E, HPC, 3, 128), np.float32)
        for h in range(HPC):
            g = core * HPC + h
            c0 = g * D
            w_all[:, h, 0, 0:64] = Wq[:, c0 + ev] * qscale
            w_all[:, h, 0, 64:128] = Wk[:, c0 + ev]
            w_all[:, h, 1, 0:64] = Wq[:, c0 + od] * qscale
            w_all[:, h, 1, 64:128] = Wk[:, c0 + od]
            w_all[:, h, 2, :] = Wv[:, c0:c0 + D]
        wo_c = Wo[core * HPC * D:(core + 1) * HPC * D, :]
        b_r, l_r = core // LC, (core % LC) * 512
        pk = np.empty((6272, 512), bf)
        pk[0:2048] = xT[b_r, :, l_r:l_r + 512]
        pk[2048:2176] = tbl[core * 32:(core + 1) * 32, :].reshape(128, 512)
        pk[2176:5248] = (w_all.reshape(KT, 128, HPC, 3, 128)
                         .transpose(1, 0, 2, 3, 4).reshape(3072, 512).astype(bf))
        pk[5248:6272] = (wo_c.reshape(HPC, 128, E)
                         .transpose(1, 0, 2).reshape(1024, 512).astype(bf))
        maps.append({"pk": pk})
    return maps


def kernel(x, Wq, Wk, Wv, Wo, bo):
    nc = _get_program()
    maps = make_in_maps(x, Wq, Wk, Wv, Wo)
    res = run_bass_kernel_spmd(nc, maps, core_ids=list(range(NCORES)))
    y = np.concatenate(
        [np.asarray(res.results[c]["y"], np.float32) for c in range(NCORES)],
        axis=0).reshape(B, L, E)
    y += np.asarray(bo, np.float32)[None, None, :]
    return y.astype(np.float32)


# revision 8
# speedup vs baseline: 3.4256x; 1.5766x over previous
"""Causal self-attention (B=2, L=2048, E=2048, H=16, D=128) on 8 trn2 cores.

Sharding: Megatron-style tensor parallel over heads (2 heads/core) with
minimal host<->device traffic:
  - x is sharded: core r receives xT slice [E, 512] for (b=r//4, l-block r%4);
    an on-device AllGather rebuilds the full activation in device DRAM.
  - RoPE sin/cos tables are sharded 1/8 per core and AllGathered on device.
  - Causal masks are built on device with affine_select (no wire traffic).
  - Each core computes a full [B*L, E] partial of the output projection; an
    on-device ReduceScatter sums partials so each core returns only its
    [512, E] slice.  Host concatenates slices and adds the bias.

Device kernel per core, per batch:
  - qT/kT [D=128, L] for both heads via weight-stationary matmuls streaming
    the AllGathered xT once per batch (host pre-permutes Wq/Wk columns to
    (evens, odds) so RoPE is a handful of wide DVE/GpSimd ops).
  - attention works on transposed scores: sT[k, q] = kT.T @ qT needs no
    DMA-transposes at all; exp(sT) chunks feed attn@V (N=512 matmuls) and a
    ones-vector matmul that accumulates softmax denominators; the final
    normalization multiplies attn@V output by a PE-broadcast reciprocal.
  - out-projection contracts over D with per-head stationary tiles, writes
    bf16 partials to DRAM for the ReduceScatter.
"""

import os

import numpy as np
import ml_dtypes

import concourse.bass as bass
import concourse.tile as tile
from concourse import bacc, mybir
from concourse.bass_utils import run_bass_kernel_spmd

BF16 = mybir.dt.bfloat16
F32 = mybir.dt.float32
AF = mybir.ActivationFunctionType
ALU = mybir.AluOpType

B, L, E = 2, 2048, 2048
H, D = 16, 128
NCORES = 8
HPC = H // NCORES          # heads per core
KT = E // 128              # 16 contraction tiles
LC = L // 512              # 4 column chunks of L per batch
QT = L // 128              # 16 q tiles
QB = L // 512              # 4 q blocks of 512
THETA = 10000.0
NEG = -1.0e30

_PROG = None


def _build_program():
    nc = bacc.Bacc("TRN2", target_bir_lowering=False, debug=False,
                   enable_asserts=False, num_devices=NCORES)

    # single packed input: rows [0:2048] xT slice, [2048:2176] rope-table
    # shard, [2176:5248] qkv weights ([p, kt, h, j, d] order), [5248:6272]
    # out-proj weights ([p, h, e] order)
    pk_d = nc.dram_tensor("pk", [6272, 512], BF16, kind="ExternalInput").ap()
    y_d = nc.dram_tensor("y", [B * L // NCORES, E], BF16, kind="ExternalOutput").ap()

    grp = [list(range(NCORES))]

    with tile.TileContext(nc) as tc:
        with tc.tile_pool(name="consts", bufs=1) as cpool, \
             tc.tile_pool(name="xt", bufs=8) as xpool, \
             tc.tile_pool(name="rope", bufs=6) as rpool, \
             tc.tile_pool(name="qkv", bufs=4) as qkvpool, \
             tc.tile_pool(name="pp", bufs=8) as ppool, \
             tc.tile_pool(name="small", bufs=8) as spool, \
             tc.tile_pool(name="pacc", bufs=4) as papool, \
             tc.tile_pool(name="rbs", bufs=2) as rbpool, \
             tc.tile_pool(name="ys", bufs=2) as ypool, \
             tc.tile_pool(name="ps", bufs=8, space="PSUM") as pspool, \
             tc.tile_pool(name="dram", bufs=1, space="DRAM") as dram:

            # ---- weights ----
            w_sb = cpool.tile([128, KT, HPC, 3, 128], BF16, tag="w")
            nc.sync.dma_start(
                w_sb[:], pk_d[2176:5248, :].rearrange("(p r) c -> p (r c)", p=128))
            wo_sb = cpool.tile([128, HPC, E], BF16, tag="wo")
            nc.sync.dma_start(
                wo_sb[:], pk_d[5248:6272, :].rearrange("(p r) c -> p (r c)", p=128))

            # ---- RoPE tables: AllGather 1/8 shards, convert to f32 ----
            tb_b = dram.tile([32, L], BF16, tag="tb_b")
            tblg = dram.tile([32 * NCORES, L], BF16, tag="tblg")
            nc.sync.dma_start(tb_b[:], pk_d[2048:2176, :])
            nc.gpsimd.collective_compute(
                "AllGather", ALU.bypass, replica_groups=grp,
                ins=[tb_b.opt()], outs=[tblg.opt()])
            cs = cpool.tile([128, L], F32, tag="cs")
            ss = cpool.tile([128, L], F32, tag="ss")
            for dst, r0 in ((cs, 0), (ss, 128)):
                for c4 in range(4):
                    tmp = xpool.tile([128, 512], BF16, tag="xt",
                                     name=f"tbl_{r0}_{c4}")
                    nc.sync.dma_start(tmp[:], tblg[r0:r0 + 128,
                                                   c4 * 512:(c4 + 1) * 512])
                    nc.vector.tensor_copy(dst[:, c4 * 512:(c4 + 1) * 512], tmp[:])

            # ---- causal masks for the 4 diagonal chunk offsets ----
            masks = []
            for d in range(4):
                m = cpool.tile([128, 512], F32, tag=f"mask{d}")
                nc.gpsimd.memset(m[:], 0.0)
                # keep 0 where (col - part - 128*d) >= 0, else -1e30
                nc.gpsimd.affine_select(
                    m[:], m[:], pattern=[[1, 512]], compare_op=ALU.is_ge,
                    fill=NEG, base=-128 * d, channel_multiplier=-1)
                masks.append(m)

            ones_col = cpool.tile([128, 1], BF16, tag="ones_col")
            nc.gpsimd.memset(ones_col[:], 1.0)
            ones_row = cpool.tile([1, 128], BF16, tag="ones_row")
            nc.gpsimd.memset(ones_row[:], 1.0)

            for rep in range(int(os.environ.get("KREP", "1"))):
                # ---- AllGather x shards ----
                xs_b = dram.tile([E, 512], BF16, tag="xs_b")
                xg = dram.tile([NCORES * E, 512], BF16, tag="xg")
                nc.sync.dma_start(xs_b[:], pk_d[0:E, :])
                nc.gpsimd.collective_compute(
                    "AllGather", ALU.bypass, replica_groups=grp,
                    ins=[xs_b.opt()], outs=[xg.opt()])

                ypart = dram.tile([B * L, E], BF16, tag="ypart")

                for b in range(B):
                    # ---- QKV projection + RoPE for both heads ----
                    qT = [qkvpool.tile([128, L], BF16, tag="qT",
                                       name=f"qT_b{b}h{h}") for h in range(HPC)]
                    kT = [qkvpool.tile([128, L], BF16, tag="kT",
                                       name=f"kT_b{b}h{h}") for h in range(HPC)]
                    vTs = [qkvpool.tile([128, L], BF16, tag="vTs",
                                        name=f"vTs_b{b}h{h}") for h in range(HPC)]
                    for lc in range(LC):
                        ls = lc * 512
                        pA = [pspool.tile([128, 512], F32, tag="ps",
                                          name=f"pA{b}_{lc}_{h}") for h in range(HPC)]
                        pB = [pspool.tile([128, 512], F32, tag="ps",
                                          name=f"pB{b}_{lc}_{h}") for h in range(HPC)]
                        pV = [pspool.tile([128, 512], F32, tag="ps",
                                          name=f"pV{b}_{lc}_{h}") for h in range(HPC)]
                        for kt in range(KT):
                            xt = xpool.tile([128, 512], BF16, tag="xt")
                            row0 = (b * LC + lc) * E + kt * 128
                            nc.sync.dma_start(xt[:], xg[row0:row0 + 128, :])
                            st = kt == 0
                            sp = kt == KT - 1
                            for h in range(HPC):
                                nc.tensor.matmul(pA[h][:], w_sb[:, kt, h, 0, :], xt[:],
                                                 start=st, stop=sp)
                                nc.tensor.matmul(pB[h][:], w_sb[:, kt, h, 1, :], xt[:],
                                                 start=st, stop=sp)
                                nc.tensor.matmul(pV[h][:], w_sb[:, kt, h, 2, :], xt[:],
                                                 start=st, stop=sp)
                        for h in range(HPC):
                            # RoPE: rows of A/B are [q-even|k-even] / [q-odd|k-odd]
                            t1 = rpool.tile([128, 512], F32, tag="rt")
                            nc.vector.tensor_mul(t1[:], pA[h][:], cs[:, ls:ls + 512])
                            t2 = rpool.tile([128, 512], F32, tag="rt")
                            nc.vector.tensor_mul(t2[:], pB[h][:], ss[:, ls:ls + 512])
                            t3 = rpool.tile([128, 512], F32, tag="rt")
                            nc.vector.tensor_mul(t3[:], pA[h][:], ss[:, ls:ls + 512])
                            t4 = rpool.tile([128, 512], F32, tag="rt")
                            nc.vector.tensor_mul(t4[:], pB[h][:], cs[:, ls:ls + 512])
                            nc.vector.tensor_sub(qT[h][0:64, ls:ls + 512],
                                                 t1[0:64, :], t2[0:64, :])
                            nc.vector.tensor_sub(kT[h][0:64, ls:ls + 512],
                                                 t1[64:128, :], t2[64:128, :])
                            nc.vector.tensor_add(qT[h][64:128, ls:ls + 512],
                                                 t3[0:64, :], t4[0:64, :])
                            nc.vector.tensor_add(kT[h][64:128, ls:ls + 512],
                                                 t3[64:128, :], t4[64:128, :])
                            nc.scalar.copy(vTs[h][:, ls:ls + 512], pV[h][:])

                    vN = [qkvpool.tile([128, KT, 128], BF16, tag="vN",
                                       name=f"vN_b{b}h{h}") for h in range(HPC)]
                    for h in range(HPC):
                        nc.scalar.dma_start_transpose(out=vN[h][:], in_=vTs[h][:])

                    # ---- attention (transposed scores; no P transposes) ----
                    outT = [qkvpool.tile([128, L], BF16, tag="oT",
                                         name=f"oT_b{b}h{h}") for h in range(HPC)]
                    for h in range(HPC):
                        for qb in range(QB):
                            qs = qb * 512
                            nch = 4 * (qb + 1)
                            pv = pspool.tile([128, 512], F32, tag="ps")
                            pacc = papool.tile([128, 512], BF16, tag="pacc",
                                              name=f"pacc{b}_{h}_{qb}")
                            for kb in range(nch):
                                s = pspool.tile([128, 512], F32, tag="ps")
                                nc.tensor.matmul(
                                    s[:], kT[h][:, kb * 128:(kb + 1) * 128],
                                    qT[h][:, qs:qs + 512], start=True, stop=True)
                                dd = kb - 4 * qb
                                if dd >= 0:
                                    nc.vector.tensor_tensor(
                                        s[:], s[:], masks[dd][:], op=ALU.add)
                                pt = ppool.tile([128, 512], BF16, tag="pt")
                                nc.scalar.activation(pt[:], s[:], AF.Exp)
                                if kb == 0:
                                    nc.vector.tensor_copy(pacc[:], pt[:])
                                else:
                                    nc.vector.tensor_add(pacc[:], pacc[:], pt[:])
                                nc.tensor.matmul(pv[:], vN[h][:, kb, :], pt[:],
                                                 start=(kb == 0),
                                                 stop=(kb == nch - 1))
                            dn = pspool.tile([1, 512], F32, tag="ps")
                            nc.tensor.matmul(dn[:], ones_col[:], pacc[:],
                                             start=True, stop=True)
                            rinv = spool.tile([1, 512], F32, tag="ri")
                            nc.vector.reciprocal(rinv[:], dn[:])
                            rinv_h = spool.tile([1, 512], BF16, tag="rih")
                            nc.vector.tensor_copy(rinv_h[:], rinv[:])
                            rb = pspool.tile([128, 512], F32, tag="ps")
                            nc.tensor.matmul(rb[:], ones_row[:], rinv_h[:],
                                             start=True, stop=True)
                            rbs = rbpool.tile([128, 512], F32, tag="rbs")
                            nc.scalar.copy(rbs[:], rb[:])
                            nc.vector.tensor_mul(outT[h][:, qs:qs + 512],
                                                 pv[:], rbs[:])

                    # ---- output projection (partial over this core's heads) ----
                    for qt in range(QT):
                        ysb = ypool.tile([128, E], BF16, tag="ysb")
                        qs = qt * 128
                        yp = [pspool.tile([128, 512], F32, tag="ps",
                                          name=f"yp{b}_{qt}_{ec}") for ec in range(4)]
                        for h in range(HPC):
                            for ec in range(4):
                                nc.tensor.matmul(
                                    yp[ec][:], outT[h][:, qs:qs + 128],
                                    wo_sb[:, h, ec * 512:(ec + 1) * 512],
                                    start=(h == 0), stop=(h == HPC - 1))
                        for ec in range(4):
                            es = ec * 512
                            if ec % 2 == 0:
                                nc.scalar.copy(ysb[:, es:es + 512], yp[ec][:])
                            else:
                                nc.vector.tensor_copy(ysb[:, es:es + 512], yp[ec][:])
                        nc.sync.dma_start(
                            ypart[b * L + qs:b * L + qs + 128, :], ysb[:])

                # ---- ReduceScatter partials; write this core's slice ----
                ysl = dram.tile([B * L // NCORES, E], BF16, tag="ysl")
                nc.gpsimd.collective_compute(
                    "ReduceScatter", ALU.add, replica_groups=grp,
                    ins=[ypart.opt()], outs=[ysl.opt()])
                nc.sync.dma_start(y_d[:], ysl[:])

    nc.compile()
    return nc


def _get_program():
    global _PROG
    if _PROG is None:
        _PROG = _build_program()
    return _PROG


def make_in_maps(x, Wq, Wk, Wv, Wo):
    """Host-side sharding/layout prep. Returns list of 8 per-core input maps."""
    bf = ml_dtypes.bfloat16
    x = np.asarray(x, np.float32)
    Wq = np.asarray(Wq, np.float32)
    Wk = np.asarray(Wk, np.float32)
    Wv = np.asarray(Wv, np.float32)
    Wo = np.asarray(Wo, np.float32)

    xT = np.ascontiguousarray(x.transpose(0, 2, 1)).astype(bf)  # [B, E, L]

    inv = THETA ** (-np.arange(0, D, 2, dtype=np.float32) / D)  # [64]
    ang = np.arange(L, dtype=np.float32)[:, None] * inv[None, :]  # [L, 64]
    cosf = np.concatenate([np.cos(ang).T] * 2, axis=0)  # [128, L]
    sinf = np.concatenate([np.sin(ang).T] * 2, axis=0)
    tbl = np.ascontiguousarray(
        np.concatenate([cosf, sinf], axis=0)).astype(bf)  # [256, L]

    qscale = np.float32(D ** -0.5)
    ev = np.arange(0, D, 2)
    od = np.arange(1, D, 2)

    maps = []
    for core in range(NCORES):
        w_all = np.empty((E, HPC, 3, 128), np.float32)
        for h in range(HPC):
            g = core * HPC + h
            c0 = g * D
            w_all[:, h, 0, 0:64] = Wq[:, c0 + ev] * qscale
            w_all[:, h, 0, 64:128] = Wk[:, c0 + ev]
            w_all[:, h, 1, 0:64] = Wq[:, c0 + od] * qscale
            w_all[:, h, 1, 64:128] = Wk[:, c0 + od]
            w_all[:, h, 2, :] = Wv[:, c0:c0 + D]
        wo_c = Wo[core * HPC * D:(core + 1) * HPC * D, :]
        b_r, l_r = core // LC, (core % LC) * 512
        pk = np.empty((6272, 512), bf)
        pk[0:2048] = xT[b_r, :, l_r:l_r + 512]
        pk[2048:2176] = tbl[core * 32:(core + 1) * 32, :].reshape(128, 512)
        pk[2176:5248] = (w_all.reshape(KT, 128, HPC, 3, 128)
                         .transpose(1, 0, 2, 3, 4).reshape(3072, 512).astype(bf))
        pk[5248:6272] = (wo_c.reshape(HPC, 128, E)
                         .transpose(1, 0, 2).reshape(1024, 512).astype(bf))
        maps.append({"pk": pk})
    return maps


def kernel(x, Wq, Wk, Wv, Wo, bo):
    nc = _get_program()
    maps = make_in_maps(x, Wq, Wk, Wv, Wo)
    res = run_bass_kernel_spmd(nc, maps, core_ids=list(range(NCORES)))
    y = np.concatenate(
        [np.asarray(res.results[c]["y"], np.float32) for c in range(NCORES)],
        axis=0).reshape(B, L, E)
    y += np.asarray(bo, np.float32)[None, None, :]
    return y.astype(np.float32)


# revision 11
# speedup vs baseline: 3.7175x; 1.0852x over previous
"""Causal self-attention (B=2, L=2048, E=2048, H=16, D=128) on 8 trn2 cores.

Sharding: Megatron-style tensor parallel over heads (2 heads/core) with
minimal host<->device traffic:
  - x is sharded: core r receives xT slice [E, 512] for (b=r//4, l-block r%4);
    an on-device AllGather rebuilds the full activation in device DRAM.
  - RoPE sin/cos tables are sharded 1/8 per core and AllGathered on device.
  - Causal masks are built on device with affine_select (no wire traffic).
  - Each core computes a full [B*L, E] partial of the output projection; an
    on-device ReduceScatter sums partials so each core returns only its
    [512, E] slice.  Host concatenates slices and adds the bias.

Device kernel per core, per batch:
  - qT/kT [D=128, L] for both heads via weight-stationary matmuls streaming
    the AllGathered xT once per batch (host pre-permutes Wq/Wk columns to
    (evens, odds) so RoPE is a handful of wide DVE/GpSimd ops).
  - attention works on transposed scores: sT[k, q] = kT.T @ qT needs no
    DMA-transposes at all; exp(sT) chunks feed attn@V (N=512 matmuls) and a
    ones-vector matmul that accumulates softmax denominators; the final
    normalization multiplies attn@V output by a PE-broadcast reciprocal.
  - out-projection contracts over D with per-head stationary tiles, writes
    bf16 partials to DRAM for the ReduceScatter.
"""

import os

import numpy as np
import ml_dtypes

import concourse.bass as bass
import concourse.tile as tile
from concourse import bacc, mybir
from concourse.bass_utils import run_bass_kernel_spmd

BF16 = mybir.dt.bfloat16
F32 = mybir.dt.float32
AF = mybir.ActivationFunctionType
ALU = mybir.AluOpType

B, L, E = 2, 2048, 2048
H, D = 16, 128
NCORES = 8
HPC = H // NCORES          # heads per core
KT = E // 128              # 16 contraction tiles
LC = L // 512              # 4 column chunks of L per batch
QT = L // 128              # 16 q tiles
QB = L // 512              # 4 q blocks of 512
THETA = 10000.0
NEG = -1.0e30

_PROG = None


def _build_program():
    nc = bacc.Bacc("TRN2", target_bir_lowering=False, debug=False,
                   enable_asserts=False, num_devices=NCORES)

    # single packed input: rows [0:2048] xT slice, [2048:2176] rope-table
    # shard, [2176:5248] qkv weights ([p, kt, h, j, d] order), [5248:6272]
    # out-proj weights ([p, h, e] order)
    pk_d = nc.dram_tensor("pk", [6272, 512], BF16, kind="ExternalInput").ap()
    y_d = nc.dram_tensor("y", [B * L // NCORES, E], BF16, kind="ExternalOutput").ap()

    grp = [list(range(NCORES))]

    with tile.TileContext(nc) as tc:
        with tc.tile_pool(name="consts", bufs=1) as cpool, \
             tc.tile_pool(name="xt", bufs=8) as xpool, \
             tc.tile_pool(name="rope", bufs=6) as rpool, \
             tc.tile_pool(name="qkv", bufs=4) as qkvpool, \
             tc.tile_pool(name="pp", bufs=8) as ppool, \
             tc.tile_pool(name="small", bufs=8) as spool, \
             tc.tile_pool(name="pacc", bufs=4) as papool, \
             tc.tile_pool(name="rbs", bufs=2) as rbpool, \
             tc.tile_pool(name="ys", bufs=2) as ypool, \
             tc.tile_pool(name="ps", bufs=8, space="PSUM") as pspool, \
             tc.tile_pool(name="dram", bufs=1, space="DRAM") as dram:

            # ---- weights ----
            w_sb = cpool.tile([128, KT, HPC, 3, 128], BF16, tag="w")
            nc.sync.dma_start(
                w_sb[:], pk_d[2176:5248, :].rearrange("(p r) c -> p (r c)", p=128))
            wo_sb = cpool.tile([128, HPC, E], BF16, tag="wo")
            nc.sync.dma_start(
                wo_sb[:], pk_d[5248:6272, :].rearrange("(p r) c -> p (r c)", p=128))


            # ---- causal masks for the 4 diagonal chunk offsets ----
            masks = []
            for d in range(4):
                m = cpool.tile([128, 512], F32, tag=f"mask{d}")
                nc.gpsimd.memset(m[:], 0.0)
                # keep 0 where (col - part - 128*d) >= 0, else -1e30
                nc.gpsimd.affine_select(
                    m[:], m[:], pattern=[[1, 512]], compare_op=ALU.is_ge,
                    fill=NEG, base=-128 * d, channel_multiplier=-1)
                masks.append(m)

            ones_col = cpool.tile([128, 1], BF16, tag="ones_col")
            nc.gpsimd.memset(ones_col[:], 1.0)
            ones_row = cpool.tile([1, 128], BF16, tag="ones_row")
            nc.gpsimd.memset(ones_row[:], 1.0)

            # ---- RoPE tables: AllGather 1/8 shards, convert to f32 ----
            tb_b = dram.tile([32, L], BF16, tag="tb_b")
            tblg = dram.tile([32 * NCORES, L], BF16, tag="tblg")
            nc.sync.dma_start(tb_b[:], pk_d[2048:2176, :])
            nc.gpsimd.collective_compute(
                "AllGather", ALU.bypass, replica_groups=grp,
                ins=[tb_b.opt()], outs=[tblg.opt()])
            cs = cpool.tile([128, L], F32, tag="cs")
            ss = cpool.tile([128, L], F32, tag="ss")
            for dst, r0 in ((cs, 0), (ss, 128)):
                for c4 in range(4):
                    tmp = xpool.tile([128, 512], BF16, tag="xt",
                                     name=f"tbl_{r0}_{c4}")
                    nc.sync.dma_start(tmp[:], tblg[r0:r0 + 128,
                                                   c4 * 512:(c4 + 1) * 512])
                    nc.vector.tensor_copy(dst[:, c4 * 512:(c4 + 1) * 512], tmp[:])

            for rep in range(int(os.environ.get("KREP", "1"))):
                # ---- AllGather x shards ----
                xs_b = dram.tile([E, 512], BF16, tag="xs_b")
                xg = dram.tile([NCORES * E, 512], BF16, tag="xg")
                nc.sync.dma_start(xs_b[:], pk_d[0:E, :])
                nc.gpsimd.collective_compute(
                    "AllGather", ALU.bypass, replica_groups=grp,
                    ins=[xs_b.opt()], outs=[xg.opt()])

                ypart = dram.tile([B * L, E], BF16, tag="ypart")

                for b in range(B):
                    # ---- QKV projection + RoPE for both heads ----
                    qT = [qkvpool.tile([128, L], BF16, tag="qT",
                                       name=f"qT_b{b}h{h}") for h in range(HPC)]
                    kT = [qkvpool.tile([128, L], BF16, tag="kT",
                                       name=f"kT_b{b}h{h}") for h in range(HPC)]
                    vTs = [qkvpool.tile([128, L], BF16, tag="vTs",
                                        name=f"vTs_b{b}h{h}") for h in range(HPC)]
                    for lc in range(LC):
                        ls = lc * 512
                        pA = [pspool.tile([128, 512], F32, tag="ps",
                                          name=f"pA{b}_{lc}_{h}") for h in range(HPC)]
                        pB = [pspool.tile([128, 512], F32, tag="ps",
                                          name=f"pB{b}_{lc}_{h}") for h in range(HPC)]
                        pV = [pspool.tile([128, 512], F32, tag="ps",
                                          name=f"pV{b}_{lc}_{h}") for h in range(HPC)]
                        for kt in range(KT):
                            xt = xpool.tile([128, 512], BF16, tag="xt")
                            row0 = (b * LC + lc) * E + kt * 128
                            nc.sync.dma_start(xt[:], xg[row0:row0 + 128, :])
                            st = kt == 0
                            sp = kt == KT - 1
                            for h in range(HPC):
                                nc.tensor.matmul(pA[h][:], w_sb[:, kt, h, 0, :], xt[:],
                                                 start=st, stop=sp)
                                nc.tensor.matmul(pB[h][:], w_sb[:, kt, h, 1, :], xt[:],
                                                 start=st, stop=sp)
                                nc.tensor.matmul(pV[h][:], w_sb[:, kt, h, 2, :], xt[:],
                                                 start=st, stop=sp)
                        for h in range(HPC):
                            # RoPE: rows of A/B are [q-even|k-even] / [q-odd|k-odd]
                            t1 = rpool.tile([128, 512], F32, tag="rt")
                            nc.vector.tensor_mul(t1[:], pA[h][:], cs[:, ls:ls + 512])
                            t2 = rpool.tile([128, 512], F32, tag="rt")
                            nc.vector.tensor_mul(t2[:], pB[h][:], ss[:, ls:ls + 512])
                            t3 = rpool.tile([128, 512], F32, tag="rt")
                            nc.vector.tensor_mul(t3[:], pA[h][:], ss[:, ls:ls + 512])
                            t4 = rpool.tile([128, 512], F32, tag="rt")
                            nc.vector.tensor_mul(t4[:], pB[h][:], cs[:, ls:ls + 512])
                            nc.vector.tensor_sub(qT[h][0:64, ls:ls + 512],
                                                 t1[0:64, :], t2[0:64, :])
                            nc.vector.tensor_sub(kT[h][0:64, ls:ls + 512],
                                                 t1[64:128, :], t2[64:128, :])
                            nc.vector.tensor_add(qT[h][64:128, ls:ls + 512],
                                                 t3[0:64, :], t4[0:64, :])
                            nc.vector.tensor_add(kT[h][64:128, ls:ls + 512],
                                                 t3[64:128, :], t4[64:128, :])
                            nc.scalar.copy(vTs[h][:, ls:ls + 512], pV[h][:])

                    vN = [qkvpool.tile([128, KT, 128], BF16, tag="vN",
                                       name=f"vN_b{b}h{h}") for h in range(HPC)]
                    for h in range(HPC):
                        nc.scalar.dma_start_transpose(out=vN[h][:], in_=vTs[h][:])

                    # ---- attention (transposed scores; no P transposes) ----
                    outT = [qkvpool.tile([128, L], BF16, tag="oT",
                                         name=f"oT_b{b}h{h}") for h in range(HPC)]
                    for h in range(HPC):
                        for qb in range(QB):
                            qs = qb * 512
                            nch = 4 * (qb + 1)
                            pv = pspool.tile([128, 512], F32, tag="ps")
                            pacc = papool.tile([128, 512], BF16, tag="pacc",
                                              name=f"pacc{b}_{h}_{qb}")
                            for kb in range(nch):
                                s = pspool.tile([128, 512], F32, tag="ps")
                                nc.tensor.matmul(
                                    s[:], kT[h][:, kb * 128:(kb + 1) * 128],
                                    qT[h][:, qs:qs + 512], start=True, stop=True)
                                dd = kb - 4 * qb
                                if dd >= 0:
                                    nc.vector.tensor_tensor(
                                        s[:], s[:], masks[dd][:], op=ALU.add)
                                pt = ppool.tile([128, 512], BF16, tag="pt")
                                nc.scalar.activation(pt[:], s[:], AF.Exp)
                                if kb == 0:
                                    nc.vector.tensor_copy(pacc[:], pt[:])
                                else:
                                    nc.vector.tensor_add(pacc[:], pacc[:], pt[:])
                                nc.tensor.matmul(pv[:], vN[h][:, kb, :], pt[:],
                                                 start=(kb == 0),
                                                 stop=(kb == nch - 1))
                            dn = pspool.tile([1, 512], F32, tag="ps")
                            nc.tensor.matmul(dn[:], ones_col[:], pacc[:],
                                             start=True, stop=True)
                            rinv = spool.tile([1, 512], F32, tag="ri")
                            nc.vector.reciprocal(rinv[:], dn[:])
                            rinv_h = spool.tile([1, 512], BF16, tag="rih")
                            nc.vector.tensor_copy(rinv_h[:], rinv[:])
                            rb = pspool.tile([128, 512], F32, tag="ps")
                            nc.tensor.matmul(rb[:], ones_row[:], rinv_h[:],
                                             start=True, stop=True)
                            rbs = rbpool.tile([128, 512], F32, tag="rbs")
                            nc.scalar.copy(rbs[:], rb[:])
                            nc.vector.tensor_mul(outT[h][:, qs:qs + 512],
                                                 pv[:], rbs[:])

                    # ---- output projection (partial over this core's heads) ----
                    for qt in range(QT):
                        ysb = ypool.tile([128, E], BF16, tag="ysb")
                        qs = qt * 128
                        yp = [pspool.tile([128, 512], F32, tag="ps",
                                          name=f"yp{b}_{qt}_{ec}") for ec in range(4)]
                        for h in range(HPC):
                            for ec in range(4):
                                nc.tensor.matmul(
                                    yp[ec][:], outT[h][:, qs:qs + 128],
                                    wo_sb[:, h, ec * 512:(ec + 1) * 512],
                                    start=(h == 0), stop=(h == HPC - 1))
                        for ec in range(4):
                            es = ec * 512
                            if ec % 2 == 0:
                                nc.scalar.copy(ysb[:, es:es + 512], yp[ec][:])
                            else:
                                nc.vector.tensor_copy(ysb[:, es:es + 512], yp[ec][:])
                        nc.sync.dma_start(
                            ypart[b * L + qs:b * L + qs + 128, :], ysb[:])

                # ---- ReduceScatter partials; write this core's slice ----
                ysl = dram.tile([B * L // NCORES, E], BF16, tag="ysl")
                nc.gpsimd.collective_compute(
                    "ReduceScatter", ALU.add, replica_groups=grp,
                    ins=[ypart.opt()], outs=[ysl.opt()])
                nc.sync.dma_start(y_d[:], ysl[:])

    nc.compile()
    return nc


def _get_program():
    global _PROG
    if _PROG is None:
        _PROG = _build_program()
    return _PROG


def make_in_maps(x, Wq, Wk, Wv, Wo):
    """Host-side sharding/layout prep. Returns list of 8 per-core input maps."""
    bf = ml_dtypes.bfloat16
    x = np.asarray(x, np.float32)
    Wq = np.asarray(Wq, np.float32)
    Wk = np.asarray(Wk, np.float32)
    Wv = np.asarray(Wv, np.float32)
    Wo = np.asarray(Wo, np.float32)

    xT = np.ascontiguousarray(x.transpose(0, 2, 1)).astype(bf)  # [B, E, L]

    inv = THETA ** (-np.arange(0, D, 2, dtype=np.float32) / D)  # [64]
    ang = np.arange(L, dtype=np.float32)[:, None] * inv[None, :]  # [L, 64]
    cosf = np.concatenate([np.cos(ang).T] * 2, axis=0)  # [128, L]
    sinf = np.concatenate([np.sin(ang).T] * 2, axis=0)
    tbl = np.ascontiguousarray(
        np.concatenate([cosf, sinf], axis=0)).astype(bf)  # [256, L]

    qscale = np.float32(D ** -0.5)
    ev = np.arange(0, D, 2)
    od = np.arange(1, D, 2)

    maps = []
    for core in range(NCORES):
        w_all = np.empty((E, HPC, 3, 128), np.float32)
        for h in range(HPC):
            g = core * HPC + h
            c0 = g * D
            w_all[:, h, 0, 0:64] = Wq[:, c0 + ev] * qscale
            w_all[:, h, 0, 64:128] = Wk[:, c0 + ev]
            w_all[:, h, 1, 0:64] = Wq[:, c0 + od] * qscale
            w_all[:, h, 1, 64:128] = Wk[:, c0 + od]
            w_all[:, h, 2, :] = Wv[:, c0:c0 + D]
        wo_c = Wo[core * HPC * D:(core + 1) * HPC * D, :]
        b_r, l_r = core // LC, (core % LC) * 512
        pk = np.empty((6272, 512), bf)
        pk[0:2048] = xT[b_r, :, l_r:l_r + 512]
        pk[2048:2176] = tbl[core * 32:(core + 1) * 32, :].reshape(128, 512)
        pk[2176:5248] = (w_all.reshape(KT, 128, HPC, 3, 128)
                         .transpose(1, 0, 2, 3, 4).reshape(3072, 512).astype(bf))
        pk[5248:6272] = (wo_c.reshape(HPC, 128, E)
                         .transpose(1, 0, 2).reshape(1024, 512).astype(bf))
        maps.append({"pk": pk})
    return maps


def kernel(x, Wq, Wk, Wv, Wo, bo):
    nc = _get_program()
    maps = make_in_maps(x, Wq, Wk, Wv, Wo)
    res = run_bass_kernel_spmd(nc, maps, core_ids=list(range(NCORES)))
    y = np.concatenate(
        [np.asarray(res.results[c]["y"], np.float32) for c in range(NCORES)],
        axis=0).reshape(B, L, E)
    y += np.asarray(bo, np.float32)[None, None, :]
    return y.astype(np.float32)


# revision 12
# speedup vs baseline: 4.1102x; 1.1056x over previous
"""Causal self-attention (B=2, L=2048, E=2048, H=16, D=128) on 8 trn2 cores.

Sharding: Megatron-style tensor parallel over heads (2 heads/core) with
minimal host<->device traffic:
  - x is sharded: core r receives xT slice [E, 512] for (b=r//4, l-block r%4);
    an on-device AllGather rebuilds the full activation in device DRAM.
  - RoPE sin/cos tables are sharded 1/8 per core and AllGathered on device.
  - Causal masks are built on device with affine_select (no wire traffic).
  - Each core computes a full [B*L, E] partial of the output projection; an
    on-device ReduceScatter sums partials so each core returns only its
    [512, E] slice.  Host concatenates slices and adds the bias.

Device kernel per core, per batch:
  - qT/kT [D=128, L] for both heads via weight-stationary matmuls streaming
    the AllGathered xT once per batch (host pre-permutes Wq/Wk columns to
    (evens, odds) so RoPE is a handful of wide DVE ops).
  - attention works on transposed scores: sT[k, q] = kT.T @ qT needs no
    DMA-transposes at all; exp(sT) chunks feed attn@V (N=512 matmuls) while
    DVE accumulates the chunk sum, from which one ones-vector matmul per
    q-block produces softmax denominators; the normalization multiplies
    attn@V output by a PE-broadcast reciprocal.
  - out-projection contracts over D with per-head stationary tiles, writes
    bf16 partials to DRAM for the ReduceScatter.

All inputs arrive packed in a single [6272, 512] bf16 tensor per core — the
axon execute path pays ~1 ms per IO buffer per call, so buffer count is 3
(packed input, output-init, output).
"""

import os

import numpy as np
import ml_dtypes

import concourse.bass as bass
import concourse.tile as tile
from concourse import bacc, mybir
from concourse.bass_utils import run_bass_kernel_spmd

BF16 = mybir.dt.bfloat16
F32 = mybir.dt.float32
AF = mybir.ActivationFunctionType
ALU = mybir.AluOpType

B, L, E = 2, 2048, 2048
H, D = 16, 128
NCORES = 8
HPC = H // NCORES          # heads per core
KT = E // 128              # 16 contraction tiles
LC = L // 512              # 4 column chunks of L per batch
QT = L // 128              # 16 q tiles
QB = L // 512              # 4 q blocks of 512
THETA = 10000.0
NEG = -1.0e30

_PROG = None


def _build_program():
    nc = bacc.Bacc("TRN2", target_bir_lowering=False, debug=False,
                   enable_asserts=False, num_devices=NCORES)

    # single packed input: rows [0:2048] xT slice, [2048:2176] rope-table
    # shard, [2176:5248] qkv weights ([p, kt, h, j, d] order), [5248:6272]
    # out-proj weights ([p, h, e] order)
    pk_d = nc.dram_tensor("pk", [6272, 512], BF16, kind="ExternalInput").ap()
    y_d = nc.dram_tensor("y", [B * L // NCORES, E], BF16, kind="ExternalOutput").ap()

    grp = [list(range(NCORES))]

    with tile.TileContext(nc) as tc:
        with tc.tile_pool(name="consts", bufs=1) as cpool, \
             tc.tile_pool(name="xt", bufs=8) as xpool, \
             tc.tile_pool(name="rope", bufs=6) as rpool, \
             tc.tile_pool(name="qkv", bufs=4) as qkvpool, \
             tc.tile_pool(name="pp", bufs=8) as ppool, \
             tc.tile_pool(name="small", bufs=8) as spool, \
             tc.tile_pool(name="pacc", bufs=4) as papool, \
             tc.tile_pool(name="rbs", bufs=2) as rbpool, \
             tc.tile_pool(name="ys", bufs=2) as ypool, \
             tc.tile_pool(name="ps", bufs=8, space="PSUM") as pspool, \
             tc.tile_pool(name="dram", bufs=1, space="DRAM") as dram:

            # ---- weights ----
            w_sb = cpool.tile([128, KT, HPC, 3, 128], BF16, tag="w")
            nc.sync.dma_start(
                w_sb[:], pk_d[2176:5248, :].rearrange("(p r) c -> p (r c)", p=128))
            wo_sb = cpool.tile([128, HPC, E], BF16, tag="wo")
            nc.sync.dma_start(
                wo_sb[:], pk_d[5248:6272, :].rearrange("(p r) c -> p (r c)", p=128))


            # ---- causal masks for the 4 diagonal chunk offsets ----
            masks = []
            for d in range(4):
                m = cpool.tile([128, 512], F32, tag=f"mask{d}")
                nc.gpsimd.memset(m[:], 0.0)
                # keep 0 where (col - part - 128*d) >= 0, else -1e30
                nc.gpsimd.affine_select(
                    m[:], m[:], pattern=[[1, 512]], compare_op=ALU.is_ge,
                    fill=NEG, base=-128 * d, channel_multiplier=-1)
                masks.append(m)

            ones_col = cpool.tile([128, 1], BF16, tag="ones_col")
            nc.gpsimd.memset(ones_col[:], 1.0)
            ones_row = cpool.tile([1, 128], BF16, tag="ones_row")
            nc.gpsimd.memset(ones_row[:], 1.0)

            # ---- RoPE tables: AllGather 1/8 shards, convert to f32 ----
            tb_b = dram.tile([32, L], BF16, tag="tb_b")
            tblg = dram.tile([32 * NCORES, L], BF16, tag="tblg")
            nc.sync.dma_start(tb_b[:], pk_d[2048:2176, :])
            nc.gpsimd.collective_compute(
                "AllGather", ALU.bypass, replica_groups=grp,
                ins=[tb_b.opt()], outs=[tblg.opt()])
            cs = cpool.tile([128, L], F32, tag="cs")
            ss = cpool.tile([128, L], F32, tag="ss")
            for dst, r0 in ((cs, 0), (ss, 128)):
                for c4 in range(4):
                    tmp = xpool.tile([128, 512], BF16, tag="xt",
                                     name=f"tbl_{r0}_{c4}")
                    nc.sync.dma_start(tmp[:], tblg[r0:r0 + 128,
                                                   c4 * 512:(c4 + 1) * 512])
                    nc.vector.tensor_copy(dst[:, c4 * 512:(c4 + 1) * 512], tmp[:])

            for rep in range(int(os.environ.get("KREP", "1"))):
                # ---- AllGather x shards ----
                xs_b = dram.tile([E, 512], BF16, tag="xs_b")
                xg = dram.tile([NCORES * E, 512], BF16, tag="xg")
                nc.sync.dma_start(xs_b[:], pk_d[0:E, :])
                nc.gpsimd.collective_compute(
                    "AllGather", ALU.bypass, replica_groups=grp,
                    ins=[xs_b.opt()], outs=[xg.opt()])

                ypart = dram.tile([B * L, E], BF16, tag="ypart")

                for b in range(B):
                    # ---- QKV projection + RoPE for both heads ----
                    qT = [qkvpool.tile([128, L], BF16, tag="qT",
                                       name=f"qT_b{b}h{h}") for h in range(HPC)]
                    kT = [qkvpool.tile([128, L], BF16, tag="kT",
                                       name=f"kT_b{b}h{h}") for h in range(HPC)]
                    vTs = [qkvpool.tile([128, L], BF16, tag="vTs",
                                        name=f"vTs_b{b}h{h}") for h in range(HPC)]
                    for lc in range(LC):
                        ls = lc * 512
                        pA = [pspool.tile([128, 512], F32, tag="ps",
                                          name=f"pA{b}_{lc}_{h}") for h in range(HPC)]
                        pB = [pspool.tile([128, 512], F32, tag="ps",
                                          name=f"pB{b}_{lc}_{h}") for h in range(HPC)]
                        pV = [pspool.tile([128, 512], F32, tag="ps",
                                          name=f"pV{b}_{lc}_{h}") for h in range(HPC)]
                        for kt in range(KT):
                            xt = xpool.tile([128, 512], BF16, tag="xt")
                            row0 = (b * LC + lc) * E + kt * 128
                            nc.sync.dma_start(xt[:], xg[row0:row0 + 128, :])
                            st = kt == 0
                            sp = kt == KT - 1
                            for h in range(HPC):
                                nc.tensor.matmul(pA[h][:], w_sb[:, kt, h, 0, :], xt[:],
                                                 start=st, stop=sp)
                                nc.tensor.matmul(pB[h][:], w_sb[:, kt, h, 1, :], xt[:],
                                                 start=st, stop=sp)
                                nc.tensor.matmul(pV[h][:], w_sb[:, kt, h, 2, :], xt[:],
                                                 start=st, stop=sp)
                        for h in range(HPC):
                            # RoPE: rows of A/B are [q-even|k-even] / [q-odd|k-odd]
                            t1 = rpool.tile([128, 512], F32, tag="rt")
                            nc.vector.tensor_mul(t1[:], pA[h][:], cs[:, ls:ls + 512])
                            t2 = rpool.tile([128, 512], F32, tag="rt")
                            nc.vector.tensor_mul(t2[:], pB[h][:], ss[:, ls:ls + 512])
                            t3 = rpool.tile([128, 512], F32, tag="rt")
                            nc.vector.tensor_mul(t3[:], pA[h][:], ss[:, ls:ls + 512])
                            t4 = rpool.tile([128, 512], F32, tag="rt")
                            nc.vector.tensor_mul(t4[:], pB[h][:], cs[:, ls:ls + 512])
                            nc.vector.tensor_sub(qT[h][0:64, ls:ls + 512],
                                                 t1[0:64, :], t2[0:64, :])
                            nc.vector.tensor_sub(kT[h][0:64, ls:ls + 512],
                                                 t1[64:128, :], t2[64:128, :])
                            nc.vector.tensor_add(qT[h][64:128, ls:ls + 512],
                                                 t3[0:64, :], t4[0:64, :])
                            nc.vector.tensor_add(kT[h][64:128, ls:ls + 512],
                                                 t3[64:128, :], t4[64:128, :])
                            nc.scalar.copy(vTs[h][:, ls:ls + 512], pV[h][:])

                    vN = [qkvpool.tile([128, KT, 128], BF16, tag="vN",
                                       name=f"vN_b{b}h{h}") for h in range(HPC)]
                    for h in range(HPC):
                        nc.scalar.dma_start_transpose(out=vN[h][:], in_=vTs[h][:])

                    # ---- attention (transposed scores; no P transposes) ----
                    outT = [qkvpool.tile([128, L], BF16, tag="oT",
                                         name=f"oT_b{b}h{h}") for h in range(HPC)]
                    for h in range(HPC):
                        for qb in range(QB):
                            qs = qb * 512
                            nch = 4 * (qb + 1)
                            pv = pspool.tile([128, 512], F32, tag="ps")
                            pacc = papool.tile([128, 512], BF16, tag="pacc",
                                              name=f"pacc{b}_{h}_{qb}")
                            for kb in range(nch):
                                s = pspool.tile([128, 512], F32, tag="ps")
                                nc.tensor.matmul(
                                    s[:], kT[h][:, kb * 128:(kb + 1) * 128],
                                    qT[h][:, qs:qs + 512], start=True, stop=True)
                                dd = kb - 4 * qb
                                if dd >= 0:
                                    nc.vector.tensor_tensor(
                                        s[:], s[:], masks[dd][:], op=ALU.add)
                                pt = ppool.tile([128, 512], BF16, tag="pt")
                                nc.scalar.activation(pt[:], s[:], AF.Exp)
                                if kb == 0:
                                    nc.vector.tensor_copy(pacc[:], pt[:])
                                else:
                                    nc.vector.tensor_add(pacc[:], pacc[:], pt[:])
                                nc.tensor.matmul(pv[:], vN[h][:, kb, :], pt[:],
                                                 start=(kb == 0),
                                                 stop=(kb == nch - 1))
                            dn = pspool.tile([1, 512], F32, tag="ps")
                            nc.tensor.matmul(dn[:], ones_col[:], pacc[:],
                                             start=True, stop=True)
                            rinv = spool.tile([1, 512], F32, tag="ri")
                            nc.vector.reciprocal(rinv[:], dn[:])
                            rinv_h = spool.tile([1, 512], BF16, tag="rih")
                            nc.vector.tensor_copy(rinv_h[:], rinv[:])
                            rb = pspool.tile([128, 512], F32, tag="ps")
                            nc.tensor.matmul(rb[:], ones_row[:], rinv_h[:],
                                             start=True, stop=True)
                            rbs = rbpool.tile([128, 512], F32, tag="rbs")
                            nc.scalar.copy(rbs[:], rb[:])
                            nc.vector.tensor_mul(outT[h][:, qs:qs + 512],
                                                 pv[:], rbs[:])

                    # ---- output projection (partial over this core's heads) ----
                    for qt in range(QT):
                        ysb = ypool.tile([128, E], BF16, tag="ysb")
                        qs = qt * 128
                        yp = [pspool.tile([128, 512], F32, tag="ps",
                                          name=f"yp{b}_{qt}_{ec}") for ec in range(4)]
                        for h in range(HPC):
                            for ec in range(4):
                                nc.tensor.matmul(
                                    yp[ec][:], outT[h][:, qs:qs + 128],
                                    wo_sb[:, h, ec * 512:(ec + 1) * 512],
                                    start=(h == 0), stop=(h == HPC - 1))
                        for ec in range(4):
                            es = ec * 512
                            if ec % 2 == 0:
                                nc.scalar.copy(ysb[:, es:es + 512], yp[ec][:])
                            else:
                                nc.vector.tensor_copy(ysb[:, es:es + 512], yp[ec][:])
                        nc.sync.dma_start(
                            ypart[b * L + qs:b * L + qs + 128, :], ysb[:])

                # ---- ReduceScatter partials; write this core's slice ----
                ysl = dram.tile([B * L // NCORES, E], BF16, tag="ysl")
                nc.gpsimd.collective_compute(
                    "ReduceScatter", ALU.add, replica_groups=grp,
                    ins=[ypart.opt()], outs=[ysl.opt()])
                nc.sync.dma_start(y_d[:], ysl[:])

    nc.compile()
    return nc


def _get_program():
    global _PROG
    if _PROG is None:
        _PROG = _build_program()
    return _PROG


def make_in_maps(x, Wq, Wk, Wv, Wo):
    """Host-side sharding/layout prep. Returns list of 8 per-core input maps."""
    bf = ml_dtypes.bfloat16
    x = np.asarray(x, np.float32)
    Wq = np.asarray(Wq, np.float32)
    Wk = np.asarray(Wk, np.float32)
    Wv = np.asarray(Wv, np.float32)
    Wo = np.asarray(Wo, np.float32)

    xT = np.ascontiguousarray(x.transpose(0, 2, 1)).astype(bf)  # [B, E, L]

    inv = THETA ** (-np.arange(0, D, 2, dtype=np.float32) / D)  # [64]
    ang = np.arange(L, dtype=np.float32)[:, None] * inv[None, :]  # [L, 64]
    cosf = np.concatenate([np.cos(ang).T] * 2, axis=0)  # [128, L]
    sinf = np.concatenate([np.sin(ang).T] * 2, axis=0)
    tbl = np.ascontiguousarray(
        np.concatenate([cosf, sinf], axis=0)).astype(bf)  # [256, L]

    qscale = np.float32(D ** -0.5)
    ev = np.arange(0, D, 2)
    od = np.arange(1, D, 2)

    maps = []
    for core in range(NCORES):
        w_all = np.empty((E, HPC, 3, 128), np.float32)
        for h in range(HPC):
            g = core * HPC + h
            c0 = g * D
            w_all[:, h, 0, 0:64] = Wq[:, c0 + ev] * qscale
            w_all[:, h, 0, 64:128] = Wk[:, c0 + ev]
            w_all[:, h, 1, 0:64] = Wq[:, c0 + od] * qscale
            w_all[:, h, 1, 64:128] = Wk[:, c0 + od]
            w_all[:, h, 2, :] = Wv[:, c0:c0 + D]
        wo_c = Wo[core * HPC * D:(core + 1) * HPC * D, :]
        b_r, l_r = core // LC, (core % LC) * 512
        pk = np.empty((6272, 512), bf)
        pk[0:2048] = xT[b_r, :, l_r:l_r + 512]
        pk[2048:2176] = tbl[core * 32:(core + 1) * 32, :].reshape(128, 512)
        pk[2176:5248] = (w_all.reshape(KT, 128, HPC, 3, 128)
                         .transpose(1, 0, 2, 3, 4).reshape(3072, 512).astype(bf))
        pk[5248:6272] = (wo_c.reshape(HPC, 128, E)
                         .transpose(1, 0, 2).reshape(1024, 512).astype(bf))
        maps.append({"pk": pk})
    return maps


def kernel(x, Wq, Wk, Wv, Wo, bo):
    nc = _get_program()
    maps = make_in_maps(x, Wq, Wk, Wv, Wo)
    res = run_bass_kernel_spmd(nc, maps, core_ids=list(range(NCORES)))
    y = np.concatenate(
        [np.asarray(res.results[c]["y"], np.float32) for c in range(NCORES)],
        axis=0).reshape(B, L, E)
    y += np.asarray(bo, np.float32)[None, None, :]
    return y.astype(np.float32)
